# revision 23
# baseline (speedup 1.0000x reference)
"""Bass/Tile TRN2 kernel for BasicAttention (v4).

att = softmax(tanh(hidden @ W_h.T + p_att_feats) @ W_alpha + mask) @ att_feats

Shapes: B=64, N=2048, H=1024, A=512. Data-parallel over batch across 8
NeuronCores (8 batches per core); weights replicated; no collectives.

Memory-bound: bf16-cast inputs give a ~134us/core HBM floor. Host does
layout/dtype prep plus the tiny wh = hidden @ W_h.T fold (0.006% of
FLOPs) so the device pipeline starts immediately.

Device per core, per batch b (software-pipelined):
  pa_T stream [128a, 4ab, 2048n] bf16 (1 DMA): ACT tanh with per-partition
    bias wh_T[:, b] fused -> alpha_T bf16 (no DVE add needed).
  scores on PE: lhsT = alpha_T[:, ab, c::16] stationary (M=128 regions),
    rhs = W_alpha chunk [128a, 1] -> sps[128p, c] accumulated over 4
    ablocks; ab-outer order so each chunk needs only one tanh block.
  expnorm (deferred one iteration so ACT never blocks the PE): mask add
    (DVE), exp+rowsum (ACT, bf16 out), total sum (PE ones), recip (DVE).
  att_feats stream [128p, 8c, 1024h] bf16 (2 DMAs): 32 PE matmuls
    (attn col stationary [128,1]) -> att [1,1024] PSUM f32, scale (DVE),
    store. att phases run 2-batch-skewed and PAIRED into 16us high-duty
    bursts so the PE clock-gate (HAM) warms once per pair.
"""

import numpy as np

B, N, H, A = 64, 2048, 1024, 512
NCORES = 8
BLOC = B // NCORES  # batches per core

P = 128
NT = N // P       # 16 n-columns per partition (n = p*16 + c)
AB = A // P       # 4 a-blocks
AF_SUP = 4        # att_feats columns per supertile (4 DMAs per batch)

_NC_CACHE = {}


def _build_nc():
    import concourse.bass as bass
    import concourse.mybir as mybir
    import concourse.tile as tile
    from concourse import bacc

    dt = mybir.dt
    f32, bf16 = dt.float32, dt.bfloat16
    AF = mybir.ActivationFunctionType
    OP = mybir.AluOpType

    nc = bacc.Bacc("TRN2", target_bir_lowering=False, debug=False,
                   num_devices=NCORES)

    fp8 = dt.float8e4
    paT = nc.dram_tensor("p_att_T", [BLOC, A, N], fp8, kind="ExternalInput").ap()
    af = nc.dram_tensor("att_feats", [BLOC, N, H], bf16, kind="ExternalInput").ap()
    am = nc.dram_tensor("att_masks", [BLOC, N], f32, kind="ExternalInput").ap()
    whb = nc.dram_tensor("wh_T", [A, BLOC], f32, kind="ExternalInput").ap()
    wa4 = nc.dram_tensor("W_alpha4", [P, AB], bf16, kind="ExternalInput").ap()
    out = nc.dram_tensor("att_out", [BLOC, H], f32, kind="ExternalOutput").ap()

    with tile.TileContext(nc) as tc:
        with (
            tc.tile_pool(name="consts", bufs=1) as consts,
            tc.tile_pool(name="patt", bufs=3) as pa_pool,
            tc.tile_pool(name="alpha", bufs=2) as alpha_pool,
            tc.tile_pool(name="afp", bufs=12) as af_pool,
            tc.tile_pool(name="small", bufs=6) as small,
            tc.tile_pool(name="arow", bufs=2) as arow_pool,
            tc.tile_pool(name="psmisc", bufs=2, space="PSUM") as psmisc,
            tc.tile_pool(name="psscore", bufs=2, space="PSUM") as psscore,
            tc.tile_pool(name="psatt", bufs=4, space="PSUM") as psatt,
        ):
            # ---------------- setup (tiny DMAs only) ----------------
            ones_col = consts.tile([P, 1], f32, tag="ones")
            nc.vector.memset(ones_col, 1.0)

            # wh bias [128a, 4ab, 8b] (one 16KB DMA)
            whb_sb = consts.tile([P, AB, BLOC], f32, tag="whb")
            nc.sync.dma_start(
                out=whb_sb, in_=whb.rearrange("(ab p) b -> p ab b", p=P))
            # W_alpha as [128, 4ab]
            wa_sb = consts.tile([P, AB], bf16, tag="wa")
            nc.sync.dma_start(out=wa_sb, in_=wa4)
            # all masks [128p, 8b, 16c] (one DMA)
            masks_sb = consts.tile([P, BLOC, NT], f32, tag="masks")
            nc.sync.dma_start(
                out=masks_sb, in_=am.rearrange("b (p c) -> p b c", p=P))

            # ---------------- main loop (software-pipelined) ----------------
            paT_r = [paT[b, :, :].rearrange("(ab p) n -> p ab n", p=P)
                     for b in range(BLOC)]
            af_r = [af[b, :, :].rearrange("(p c) h -> p c h", c=NT)
                    for b in range(BLOC)]

            af_tiles = {}
            sps_tiles = {}

            def patt_phase(b):
                pa_t = pa_pool.tile([P, AB, N], fp8, tag="pa", name=f"pa{b}")
                for ab in range(AB):
                    nc.sync.dma_start(out=pa_t[:, ab, :],
                                      in_=paT_r[b][:, ab, :])
                # prefetch att_feats for this batch
                tiles = []
                for st in range(NT // AF_SUP):
                    aft = af_pool.tile([P, AF_SUP, H], bf16, tag="af",
                                       name=f"af{b}_{st}")
                    nc.sync.dma_start(
                        out=aft,
                        in_=af_r[b][:, st * AF_SUP:(st + 1) * AF_SUP, :])
                    tiles.append(aft)
                af_tiles[b] = tiles

                alpha_t = alpha_pool.tile([P, AB, N], bf16, tag="alpha",
                                          name=f"alpha{b}")
                for ab in range(AB):
                    nc.scalar.activation(
                        alpha_t[:, ab, :], pa_t[:, ab, :], AF.Tanh,
                        bias=whb_sb[:, ab, b:b + 1])
                return alpha_t

            def scores_mm(b, alpha_t):
                # 64 partial scores, one single-instruction accumulation
                # group each (start=stop=True): groups never stay open, so
                # any order is legal in one PSUM bank. ab-outer order means
                # each 16-matmul pass depends on only ONE tanh ab-block, so
                # the PE never waits ~2us per block like the c-outer/
                # psum-accumulated variant did. The 4 partials per column
                # are then summed on the (idle) DVE in expnorm.
                sps = psscore.tile([P, NT, AB], f32, tag="sps",
                                   name=f"sps{b}")
                for ab in range(AB):
                    for c in range(NT):
                        # stationary = alpha_T[:, ab, c::16]  (128 n's with
                        # stride 16 -> M-dim partition p of the output)
                        nc.tensor.matmul(
                            sps[:, c, ab:ab + 1],
                            lhsT=alpha_t[:, ab, c::NT],
                            rhs=wa_sb[:, ab:ab + 1],
                            start=True, stop=True)
                sps_tiles[b] = sps

            def expnorm(b):
                sps = sps_tiles.pop(b)
                scores = small.tile([P, NT], f32, tag="scores",
                                    name=f"scores{b}")
                nc.vector.tensor_reduce(out=scores, in_=sps,
                                        axis=mybir.AxisListType.X, op=OP.add)
                nc.vector.tensor_tensor(out=scores, in0=scores,
                                        in1=masks_sb[:, b, :], op=OP.add)
                expt = small.tile([P, NT], bf16, tag="expt", name=f"expt{b}")
                rowsum = small.tile([P, 1], f32, tag="rowsum",
                                    name=f"rowsum{b}")
                nc.scalar.activation(expt, scores, AF.Exp, accum_out=rowsum)

                sum_ps = psmisc.tile([1, 1], f32, tag="mm", name=f"sum_ps{b}")
                nc.tensor.matmul(sum_ps, lhsT=rowsum, rhs=ones_col,
                                 start=True, stop=True)
                inv = small.tile([1, 1], f32, tag="inv", name=f"inv{b}")
                nc.vector.reciprocal(inv, sum_ps)
                return expt, inv

            def af_phase(b, expt, inv):
                att_lo = psatt.tile([1, A], f32, tag="att", name=f"attlo{b}")
                att_hi = psatt.tile([1, A], f32, tag="att", name=f"atthi{b}")
                for st in range(NT // AF_SUP):
                    aft = af_tiles[b][st]
                    for c in range(AF_SUP):
                        t = st * AF_SUP + c
                        lhs = expt[:, t:t + 1]
                        nc.tensor.matmul(att_lo, lhsT=lhs,
                                         rhs=aft[:, c, 0:A],
                                         start=(t == 0), stop=(t == NT - 1))
                        nc.tensor.matmul(att_hi, lhsT=lhs,
                                         rhs=aft[:, c, A:H],
                                         start=(t == 0), stop=(t == NT - 1))
                del af_tiles[b]

                att_row = arow_pool.tile([1, H], f32, tag="attrow",
                                         name=f"attrow{b}")
                nc.vector.tensor_scalar_mul(att_row[:, 0:A], att_lo, inv)
                nc.vector.tensor_scalar_mul(att_row[:, A:H], att_hi, inv)
                nc.gpsimd.dma_start(out=out[b:b + 1, :], in_=att_row)

            # Schedule: expnorm(b) deferred to iter b+1 (so exp never blocks
            # behind fresh tanh on the ACT FIFO); att phases run 2-batch
            # skewed and PAIRED into long high-duty PE bursts (HAM warmup
            # paid once per pair): iters 2:(0,) 3:(1,2) 5:(3,4) 7:(5,6)
            # end:(7,).
            att_sched = {2: (0,), 3: (1, 2), 5: (3, 4), 7: (5, 6)}
            state = {}
            for b in range(BLOC):
                # expnorm first: exp(b-1) enters the ACT FIFO ahead of
                # tanh(b) and its inputs are already ready, so neither the
                # ACT nor the dependent PE sum matmul ever stalls.
                if b >= 1:
                    state[b - 1] = expnorm(b - 1)
                alpha_t = patt_phase(b)
                for ab_ in att_sched.get(b, ()):
                    af_phase(ab_, *state.pop(ab_))
                scores_mm(b, alpha_t)
            state[BLOC - 1] = expnorm(BLOC - 1)
            af_phase(BLOC - 1, *state.pop(BLOC - 1))

    nc.compile()
    return nc


def _get_nc():
    if "nc" not in _NC_CACHE:
        _NC_CACHE["nc"] = _build_nc()
    return _NC_CACHE["nc"]


def kernel(hidden_states, att_feats, p_att_feats, att_masks, W_h, W_alpha):
    import ml_dtypes
    from concourse.bass_utils import run_bass_kernel_spmd

    nc = _get_nc()
    bf16 = ml_dtypes.bfloat16

    af16 = np.ascontiguousarray(att_feats).astype(bf16)           # [B,N,H]
    fp8 = ml_dtypes.float8_e4m3fn
    paT16 = np.ascontiguousarray(
        np.ascontiguousarray(p_att_feats).astype(fp8).transpose(0, 2, 1))
    am32 = np.ascontiguousarray(att_masks, dtype=np.float32)      # [B,N]
    hs32 = np.ascontiguousarray(hidden_states, dtype=np.float32)
    wh32 = np.ascontiguousarray(W_h, dtype=np.float32)
    whT_all = np.ascontiguousarray(wh32 @ hs32.T)                 # [A, B] f32
    wa16 = np.ascontiguousarray(
        np.asarray(W_alpha, dtype=np.float32).reshape(AB, P).T).astype(bf16)

    in_maps = []
    for i in range(NCORES):
        s = slice(i * BLOC, (i + 1) * BLOC)
        in_maps.append({
            "p_att_T": paT16[s],
            "att_feats": af16[s],
            "att_masks": am32[s],
            "wh_T": np.ascontiguousarray(whT_all[:, s]),
            "W_alpha4": wa16,
        })

    global _LAST_IN_MAPS
    _LAST_IN_MAPS = in_maps
    res = run_bass_kernel_spmd(nc, in_maps, core_ids=list(range(NCORES)))
    return np.concatenate(
        [res.results[i]["att_out"] for i in range(NCORES)], axis=0
    ).astype(np.float32)


_LAST_IN_MAPS = None


# revision 24
# speedup vs baseline: 1.1008x; 1.1008x over previous
"""Bass/Tile TRN2 kernel for BasicAttention.

att = softmax(tanh(hidden @ W_h.T + p_att_feats) @ W_alpha + mask) @ att_feats

Shapes: B=64, N=2048, H=1024, A=512. Data-parallel over batch across 8
NeuronCores (8 batches per core); weights replicated; no collectives.

Memory-bound: p_att_feats is shipped as fp8-e4m3 (empirically 7e-3 max
rel err vs the 2e-2 gate) and att_feats as bf16, dropping the per-core
HBM read stream to ~40MB (~112us at ~358GB/s). The host does
layout/dtype prep plus the tiny wh = hidden @ W_h.T fold (0.006% of the
FLOPs) so the device pipeline starts streaming immediately.

Device per core, per batch b (software-pipelined):
  pa_T stream [128a, 4ab, 2048n] fp8 (4 DMAs, subtile deps): ACT tanh
    with per-partition bias wh_T[:, b] fused -> alpha_T bf16 (no DVE
    add needed).
  scores on PE: 64 rank-128 matmuls, lhsT = alpha_T[:, ab, c::16]
    stationary (M=128 regions -> output lands directly in the n=p*16+c
    layout), rhs = W_alpha chunk [128a, 1]. Each matmul is its own
    single-instruction accumulation group (start=stop=True) writing its
    own PSUM address: any order is legal in one bank (no open zero
    region) and there is no same-address accumulate stall. ab-outer
    order so each 16-matmul pass depends on only one tanh block. The 4
    partials per column are summed on the idle DVE.
  expnorm (deferred one iteration, issued ahead of the next tanh so the
    ACT FIFO and the dependent PE sum-matmul never stall): DVE reduce +
    mask add, ACT exp (bf16 out, f32 rowsum accum), PE ones-matmul total
    sum, DVE reciprocal.
  att_feats stream [128p, 4c, 1024h] bf16 (4 DMAs): 32 PE matmuls
    (attn col stationary [128,1], F=512 lo/hi PSUM banks), scale by
    1/sum (DVE), store via gpsimd DGE. att phases run 2-batch-skewed
    and PAIRED into ~16us high-duty PE bursts so the HAM clock-gate
    warmup is paid once per pair instead of once per batch.
"""

import numpy as np

B, N, H, A = 64, 2048, 1024, 512
NCORES = 8
BLOC = B // NCORES  # batches per core

P = 128
NT = N // P       # 16 n-columns per partition (n = p*16 + c)
AB = A // P       # 4 a-blocks
AF_SUP = 4        # att_feats columns per supertile (4 DMAs per batch)

_NC_CACHE = {}


def _build_nc():
    import concourse.bass as bass
    import concourse.mybir as mybir
    import concourse.tile as tile
    from concourse import bacc

    dt = mybir.dt
    f32, bf16 = dt.float32, dt.bfloat16
    AF = mybir.ActivationFunctionType
    OP = mybir.AluOpType

    nc = bacc.Bacc("TRN2", target_bir_lowering=False, debug=False,
                   num_devices=NCORES)

    fp8 = dt.float8e4
    paT = nc.dram_tensor("p_att_T", [BLOC, A, N], fp8, kind="ExternalInput").ap()
    af = nc.dram_tensor("att_feats", [BLOC, N, H], bf16, kind="ExternalInput").ap()
    am = nc.dram_tensor("att_masks", [BLOC, N], f32, kind="ExternalInput").ap()
    whb = nc.dram_tensor("wh_T", [A, BLOC], f32, kind="ExternalInput").ap()
    wa4 = nc.dram_tensor("W_alpha4", [P, AB], bf16, kind="ExternalInput").ap()
    out = nc.dram_tensor("att_out", [BLOC, H], f32, kind="ExternalOutput").ap()

    with tile.TileContext(nc) as tc:
        with (
            tc.tile_pool(name="consts", bufs=1) as consts,
            tc.tile_pool(name="patt", bufs=3) as pa_pool,
            tc.tile_pool(name="alpha", bufs=2) as alpha_pool,
            tc.tile_pool(name="afp", bufs=12) as af_pool,
            tc.tile_pool(name="small", bufs=6) as small,
            tc.tile_pool(name="arow", bufs=2) as arow_pool,
            tc.tile_pool(name="psmisc", bufs=2, space="PSUM") as psmisc,
            tc.tile_pool(name="psscore", bufs=2, space="PSUM") as psscore,
            tc.tile_pool(name="psatt", bufs=4, space="PSUM") as psatt,
        ):
            # ---------------- setup (tiny DMAs only) ----------------
            ones_col = consts.tile([P, 1], f32, tag="ones")
            nc.vector.memset(ones_col, 1.0)

            # wh bias [128a, 4ab, 8b] (one 16KB DMA)
            whb_sb = consts.tile([P, AB, BLOC], f32, tag="whb")
            nc.sync.dma_start(
                out=whb_sb, in_=whb.rearrange("(ab p) b -> p ab b", p=P))
            # W_alpha as [128, 4ab]
            wa_sb = consts.tile([P, AB], bf16, tag="wa")
            nc.sync.dma_start(out=wa_sb, in_=wa4)
            # all masks [128p, 8b, 16c] (one DMA)
            masks_sb = consts.tile([P, BLOC, NT], f32, tag="masks")
            nc.sync.dma_start(
                out=masks_sb, in_=am.rearrange("b (p c) -> p b c", p=P))

            # ---------------- main loop (software-pipelined) ----------------
            paT_r = [paT[b, :, :].rearrange("(ab p) n -> p ab n", p=P)
                     for b in range(BLOC)]
            af_r = [af[b, :, :].rearrange("(p c) h -> p c h", c=NT)
                    for b in range(BLOC)]

            af_tiles = {}
            sps_tiles = {}

            def patt_phase(b):
                pa_t = pa_pool.tile([P, AB, N], fp8, tag="pa", name=f"pa{b}")
                for ab in range(AB):
                    nc.sync.dma_start(out=pa_t[:, ab, :],
                                      in_=paT_r[b][:, ab, :])
                # prefetch att_feats for this batch
                tiles = []
                for st in range(NT // AF_SUP):
                    aft = af_pool.tile([P, AF_SUP, H], bf16, tag="af",
                                       name=f"af{b}_{st}")
                    nc.sync.dma_start(
                        out=aft,
                        in_=af_r[b][:, st * AF_SUP:(st + 1) * AF_SUP, :])
                    tiles.append(aft)
                af_tiles[b] = tiles

                alpha_t = alpha_pool.tile([P, AB, N], bf16, tag="alpha",
                                          name=f"alpha{b}")
                for ab in range(AB):
                    nc.scalar.activation(
                        alpha_t[:, ab, :], pa_t[:, ab, :], AF.Tanh,
                        bias=whb_sb[:, ab, b:b + 1])
                return alpha_t

            def scores_mm(b, alpha_t):
                # 64 partial scores, one single-instruction accumulation
                # group each (start=stop=True): groups never stay open, so
                # any order is legal in one PSUM bank. ab-outer order means
                # each 16-matmul pass depends on only ONE tanh ab-block, so
                # the PE never waits ~2us per block like the c-outer/
                # psum-accumulated variant did. The 4 partials per column
                # are then summed on the (idle) DVE in expnorm.
                sps = psscore.tile([P, NT, AB], f32, tag="sps",
                                   name=f"sps{b}")
                for ab in range(AB):
                    for c in range(NT):
                        # stationary = alpha_T[:, ab, c::16]  (128 n's with
                        # stride 16 -> M-dim partition p of the output)
                        nc.tensor.matmul(
                            sps[:, c, ab:ab + 1],
                            lhsT=alpha_t[:, ab, c::NT],
                            rhs=wa_sb[:, ab:ab + 1],
                            start=True, stop=True)
                sps_tiles[b] = sps

            def expnorm(b):
                sps = sps_tiles.pop(b)
                scores = small.tile([P, NT], f32, tag="scores",
                                    name=f"scores{b}")
                nc.vector.tensor_reduce(out=scores, in_=sps,
                                        axis=mybir.AxisListType.X, op=OP.add)
                nc.vector.tensor_tensor(out=scores, in0=scores,
                                        in1=masks_sb[:, b, :], op=OP.add)
                expt = small.tile([P, NT], bf16, tag="expt", name=f"expt{b}")
                rowsum = small.tile([P, 1], f32, tag="rowsum",
                                    name=f"rowsum{b}")
                nc.scalar.activation(expt, scores, AF.Exp, accum_out=rowsum)

                sum_ps = psmisc.tile([1, 1], f32, tag="mm", name=f"sum_ps{b}")
                nc.tensor.matmul(sum_ps, lhsT=rowsum, rhs=ones_col,
                                 start=True, stop=True)
                inv = small.tile([1, 1], f32, tag="inv", name=f"inv{b}")
                nc.vector.reciprocal(inv, sum_ps)
                return expt, inv

            def af_phase(b, expt, inv):
                att_lo = psatt.tile([1, A], f32, tag="att", name=f"attlo{b}")
                att_hi = psatt.tile([1, A], f32, tag="att", name=f"atthi{b}")
                for st in range(NT // AF_SUP):
                    aft = af_tiles[b][st]
                    for c in range(AF_SUP):
                        t = st * AF_SUP + c
                        lhs = expt[:, t:t + 1]
                        nc.tensor.matmul(att_lo, lhsT=lhs,
                                         rhs=aft[:, c, 0:A],
                                         start=(t == 0), stop=(t == NT - 1))
                        nc.tensor.matmul(att_hi, lhsT=lhs,
                                         rhs=aft[:, c, A:H],
                                         start=(t == 0), stop=(t == NT - 1))
                del af_tiles[b]

                att_row = arow_pool.tile([1, H], f32, tag="attrow",
                                         name=f"attrow{b}")
                nc.vector.tensor_scalar_mul(att_row[:, 0:A], att_lo, inv)
                nc.vector.tensor_scalar_mul(att_row[:, A:H], att_hi, inv)
                nc.gpsimd.dma_start(out=out[b:b + 1, :], in_=att_row)

            # Schedule: expnorm(b) deferred to iter b+1 (so exp never blocks
            # behind fresh tanh on the ACT FIFO); att phases run 2-batch
            # skewed and PAIRED into long high-duty PE bursts (HAM warmup
            # paid once per pair): iters 2:(0,) 3:(1,2) 5:(3,4) 7:(5,6)
            # end:(7,).
            att_sched = {2: (0,), 3: (1, 2), 5: (3, 4), 7: (5, 6)}
            state = {}
            for b in range(BLOC):
                # expnorm first: exp(b-1) enters the ACT FIFO ahead of
                # tanh(b) and its inputs are already ready, so neither the
                # ACT nor the dependent PE sum matmul ever stalls.
                if b >= 1:
                    state[b - 1] = expnorm(b - 1)
                alpha_t = patt_phase(b)
                for ab_ in att_sched.get(b, ()):
                    af_phase(ab_, *state.pop(ab_))
                scores_mm(b, alpha_t)
            state[BLOC - 1] = expnorm(BLOC - 1)
            af_phase(BLOC - 1, *state.pop(BLOC - 1))

    nc.compile()
    return nc


def _get_nc():
    if "nc" not in _NC_CACHE:
        _NC_CACHE["nc"] = _build_nc()
    return _NC_CACHE["nc"]


def kernel(hidden_states, att_feats, p_att_feats, att_masks, W_h, W_alpha):
    import ml_dtypes
    from concourse.bass_utils import run_bass_kernel_spmd

    nc = _get_nc()
    bf16 = ml_dtypes.bfloat16

    af16 = np.ascontiguousarray(att_feats).astype(bf16)           # [B,N,H]
    fp8 = ml_dtypes.float8_e4m3fn
    paT16 = np.ascontiguousarray(
        np.ascontiguousarray(p_att_feats).astype(fp8).transpose(0, 2, 1))
    am32 = np.ascontiguousarray(att_masks, dtype=np.float32)      # [B,N]
    hs32 = np.ascontiguousarray(hidden_states, dtype=np.float32)
    wh32 = np.ascontiguousarray(W_h, dtype=np.float32)
    whT_all = np.ascontiguousarray(wh32 @ hs32.T)                 # [A, B] f32
    wa16 = np.ascontiguousarray(
        np.asarray(W_alpha, dtype=np.float32).reshape(AB, P).T).astype(bf16)

    in_maps = []
    for i in range(NCORES):
        s = slice(i * BLOC, (i + 1) * BLOC)
        in_maps.append({
            "p_att_T": paT16[s],
            "att_feats": af16[s],
            "att_masks": am32[s],
            "wh_T": np.ascontiguousarray(whT_all[:, s]),
            "W_alpha4": wa16,
        })

    global _LAST_IN_MAPS
    _LAST_IN_MAPS = in_maps
    res = run_bass_kernel_spmd(nc, in_maps, core_ids=list(range(NCORES)))
    return np.concatenate(
        [res.results[i]["att_out"] for i in range(NCORES)], axis=0
    ).astype(np.float32)


_LAST_IN_MAPS = None
